# revision 17
# baseline (speedup 1.0000x reference)
"""Trainium2 Bass kernel: scaling-and-squaring exponential of a stationary
velocity field (phi <- phi + trilinear_pull(phi, grid + phi), 8 steps, wrap).

Strategy (self-contained; shapes hardcoded for v: [2, 3, 128, 128, 128] f32):
  - 8 NeuronCores = 2 batches x 4 x-slabs (32 planes each). After each step,
    x-halo planes are exchanged with slab neighbors via an AllGather of the
    edge planes over the 4-slab replica group (masks select the two
    neighbors; the mask one-hots are a per-device host input, keeping the
    SPMD program rank-independent). No recompute halo.
  - All device tensors fp16 (DVE tensor_tensor runs 2x for 16-bit dtypes;
    fp16's 11-bit mantissa keeps the 8-step accumulated error ~4x below
    bf16). Host pre-scales v by 2^-STEPS and lays out
    [y=128(part), c=3, x(32+4), z+4(wrap)] fp16; host converts the fp16
    output back to f32.
  - Each step computes the dense masked-tap trilinear form:
      out = sum_{i,j,k} hat(dx-i)*hat(dy-j)*hat(dz-k) * phi[x+i, y+j, z+k]
    with hat(t) = relu(1-|t|) built by ScalarE activation pairs (Abs, Relu
    with affine pre-scale); the z-axis weights are written channel-expanded
    by the Relu (a stride-0 broadcast operand costs +26% on DVE TT). x/z
    taps are free-dim AP offsets; y taps load partition-shifted tiles
    straight from DRAM; odd z offsets read from a z-shifted tile copy
    (ScalarE) so every fp16 TT op stays 4B-aligned (2x mode).
"""
import numpy as np

Y = 128
Z = 128
ZP = Z + 4
STEPS = 8
HS = [1, 1, 1, 1, 1, 1, 1, 2]
SLAB = 32
XW = SLAB + 4          # owned cols at [2, 34); up to 2 halo cols each side
CHUNK_ORDER = [8, 16, 0, 24]   # middle chunks first: they depend only on
                               # owned data, giving the previous step's halo
                               # exchange a full middle-chunk window to land
                               # before the edge chunks consume it

_CACHE = {}


def _fix_multiwaits(nc):
    """This walrus accepts one sync-wait per instruction; split extras onto
    preceding same-engine NoOps."""
    from concourse import mybir
    f = nc.m.functions[0]
    for bb in f.blocks:
        il = bb.instructions
        i = 0
        while i < len(il):
            ins = il[i]
            si = getattr(ins, "sync_info", None)
            if si is None:
                i += 1
                continue
            waits = list(si.on_wait)
            if len(waits) <= 1:
                i += 1
                continue
            for k, w in enumerate(waits[:-1]):
                nop = mybir.InstNoOp(name=f"{ins.name}_w{k}", ins=[], outs=[])
                nop.engine = ins.engine
                nop.sync_info = mybir.SyncInfo(on_wait=[w], on_update=[])
                il.insert(i, nop)
                i += 1
            si.on_wait = [waits[-1]]
            i += 1


def _build_kernel(cx=8):
    from concourse import bacc, mybir, tile
    from contextlib import ExitStack
    F16 = mybir.dt.float16
    ACT = mybir.ActivationFunctionType
    nc = bacc.Bacc("TRN2", target_bir_lowering=False, debug=False, num_devices=8)

    # const APs for activation biases (hat-weight tap offsets)
    F32 = mybir.dt.float32
    for val in (-2.0, -1.0, 2.0):
        t = nc.alloc_sbuf_tensor(f"const-f32-{val}", [128, 1], F32)
        nc.gpsimd.memset(t.ap(), val)
        nc.const_aps.aps[(F32, val)] = t.ap()
    nc.all_engine_barrier()

    # host-prepared: [y, c, x(36), z(wrap-padded)], fp16, scaled 2^-8
    VD = nc.dram_tensor("v", [Y, 3, XW, ZP], F16, kind="ExternalInput")
    # per-device neighbor one-hots: [y, {left,right}, group-rank]
    NBR = nc.dram_tensor("nbr", [Y, 2, 4], F16, kind="ExternalInput")
    OUT = nc.dram_tensor("out", [Y, 3, SLAB, Z], F16, kind="ExternalOutput")

    groups = [[0, 1, 2, 3], [4, 5, 6, 7]]

    with tile.TileContext(nc) as tc, ExitStack() as stack:
        dpool = stack.enter_context(tc.tile_pool(name="dram", bufs=1, space="DRAM"))
        PB = dpool.tile([Y, 3, XW, ZP], F16, tag="pb")
        PC = dpool.tile([Y, 3, XW, ZP], F16, tag="pc")
        npool = stack.enter_context(tc.tile_pool(name="nbrp", bufs=1))
        NBRsb = npool.tile([Y, 2, 4], F16, tag="nbr")
        nc.sync.dma_start(out=NBRsb[:], in_=NBR[:])
        expool = stack.enter_context(tc.tile_pool(name="expool", bufs=1))

        bufs = [None, PB, PC]
        exch = {}

        def emit_exchange_pack(s):
            """After step s's edge chunks: ship edges into the AllGather.
            Only DMA + gpsimd instructions -- the DVE queue is untouched, so
            the collective's latency cannot head-of-line block compute."""
            hp = HS[s + 1]
            W = bufs[1 + s % 2]
            ein = dpool.tile([Y, 3, 2 * hp, ZP], F16, tag=f"ein{s}")
            eall = dpool.tile([4 * Y, 3, 2 * hp, ZP], F16, tag=f"eall{s}")
            nc.sync.dma_start(out=ein[:, :, 0:hp], in_=W[:, :, 2:2 + hp])
            nc.sync.dma_start(out=ein[:, :, hp:2 * hp],
                              in_=W[:, :, 2 + SLAB - hp:2 + SLAB])
            nc.gpsimd.collective_compute(
                "AllGather", mybir.AluOpType.bypass, replica_groups=groups,
                ins=[ein[:]], outs=[eall[:]])
            exch[s] = (hp, W, eall)

        def emit_exchange_select(s):
            """Deferred to just before the edge chunks of step s+1: load the
            gathered edges, mask-select the two neighbors, write the halos."""
            hp, W, eall = exch[s]
            for side, htag, zsl, xd in (
                    (0, "hl", slice(hp, 2 * hp), slice(2 - hp, 2)),
                    (1, "hr", slice(0, hp),
                     slice(2 + SLAB, 2 + SLAB + hp))):
                E = []
                for g in range(4):
                    e = expool.tile([Y, 3, hp, ZP], F16, tag=f"exh{g}",
                                    bufs=1, name=f"exh{g}")
                    nc.sync.dma_start(
                        out=e[:], in_=eall[g * Y:(g + 1) * Y][:, :, zsl])
                    E.append(e)
                H = expool.tile([Y, 3, hp, ZP], F16, tag=htag, bufs=1,
                                name=htag)
                for g in range(4):
                    m = NBRsb[:, side, g:g + 1]
                    if g == 0:
                        nc.vector.scalar_tensor_tensor(
                            H[:], E[g][:], m, E[g][:],
                            op0=mybir.AluOpType.mult,
                            op1=mybir.AluOpType.bypass)
                    else:
                        nc.vector.scalar_tensor_tensor(
                            H[:], E[g][:], m, H[:],
                            op0=mybir.AluOpType.mult,
                            op1=mybir.AluOpType.add)
                nc.sync.dma_start(out=W[:, :, xd], in_=H[:])

        def emit_step(s, pool, wpool, cxs, tbufs, wbufs=2, t1bufs=None,
                      kbufs=2, pre_edge=None):
            R = VD if s == 0 else bufs[1 + (s + 1) % 2]
            W = bufs[1 + s % 2]
            h = HS[s]
            last = (s == STEPS - 1)

            chunks = ([xo for xo in CHUNK_ORDER if xo < SLAB]
                      if cxs == 8 else list(range(0, SLAB, cxs)))
            for ci, xo in enumerate(chunks):
                if ci == 2 and pre_edge is not None:
                    pre_edge()
                cw = min(cxs, SLAB - xo)
                cwi = cw + 2 * h
                xb = 2 + xo - h       # input read base in buffer coords
                # ---- load y-shifted tiles; build z-shifted variants ----
                T = {}
                for j in range(-h, h + 1):
                    t0 = pool.tile([Y, 3, cwi, ZP], F16, tag=f"T{j}_0",
                                   bufs=(tbufs if abs(j) <= 1 else 1),
                                   name=f"t{j}_0")
                    if j == 0:
                        nc.sync.dma_start(out=t0[:],
                                          in_=R[:, :, xb:xb + cwi, :])
                    elif j > 0:
                        nc.sync.dma_start(out=t0[0:Y - j],
                                          in_=R[j:Y, :, xb:xb + cwi, :])
                        nc.sync.dma_start(out=t0[Y - j:Y],
                                          in_=R[0:j, :, xb:xb + cwi, :])
                    else:
                        nc.sync.dma_start(out=t0[-j:Y],
                                          in_=R[0:Y + j, :, xb:xb + cwi, :])
                        nc.sync.dma_start(out=t0[0:-j],
                                          in_=R[Y + j:Y, :, xb:xb + cwi, :])
                    t1 = pool.tile([Y, 3, cwi, ZP], F16, tag=f"T{j}_1",
                                   bufs=(t1bufs or tbufs), name=f"t{j}_1")
                    nc.scalar.copy(t1[:, :, :, 0:ZP - 1], t0[:, :, :, 1:ZP])
                    T[j] = (t0, t1)

                # ---- hat weights on ScalarE: w = relu(1 - |d - i|) ----
                T0 = T[0][0]
                WTS = {}
                for ax, axn in ((0, 'x'), (1, 'y'), (2, 'z')):
                    d = T0[:, ax, h:h + cw, 2:2 + Z]
                    for o in range(-h, h + 1):
                        ab = wpool.tile([Y, cw, Z], F16, bufs=1,
                                        tag=f"ab{axn}", name=f"ab{axn}")
                        nc.scalar.activation(ab[:], d, ACT.Abs,
                                             bias=float(-o), scale=1.0)
                        if ax == 2:
                            # expand across channels at the Relu (ScalarE is
                            # mostly idle; a stride-0 operand costs +26% on
                            # DVE TT, so the 9 consumers want a real tensor)
                            wt = wpool.tile([Y, 3, cw, Z], F16, bufs=wbufs,
                                            tag=f"w{axn}_{o}",
                                            name=f"w{axn}_{o}")
                            abb = ab[:].unsqueeze(1).broadcast_to(
                                [Y, 3, cw, Z])
                            nc.scalar.activation(wt[:], abb, ACT.Relu,
                                                 bias=1.0, scale=-1.0)
                        else:
                            wt = wpool.tile([Y, cw, Z], F16, bufs=1,
                                            tag=f"w{axn}_{o}",
                                            name=f"w{axn}_{o}")
                            nc.scalar.activation(wt[:], ab[:], ACT.Relu,
                                                 bias=1.0, scale=-1.0)
                        WTS[(ax, o)] = wt

                # ---- dense tap accumulation on DVE (all fp16, 2x) ----
                pacc = wpool.tile([Y, 3, cw, Z], F16, tag="pacc",
                                  bufs=kbufs, name="pacc")
                aij = wpool.tile([Y, 3, cw, Z], F16, bufs=kbufs,
                                 tag="aij", name="aij")
                tmp = wpool.tile([Y, 3, cw, Z], F16, bufs=kbufs,
                                 tag="tmp", name="tmp")
                wxy = wpool.tile([Y, cw, Z], F16, bufs=1,
                                 tag="wxy", name="wxy")
                first_pair = True
                for i in range(-h, h + 1):
                    for j in range(-h, h + 1):
                        for ki, k in enumerate(range(-h, h + 1)):
                            zv = (2 + k) % 2  # odd offset -> shifted tile
                            zoff = (2 + k) - zv
                            src = T[j][zv][:, :, h + i:h + i + cw,
                                           zoff:zoff + Z]
                            wzb = WTS[(2, k)][:]
                            if ki == 0:
                                nc.vector.tensor_tensor(
                                    aij[:], src, wzb, mybir.AluOpType.mult)
                            else:
                                nc.vector.tensor_tensor(
                                    tmp[:], src, wzb, mybir.AluOpType.mult)
                                nc.vector.tensor_tensor(
                                    aij[:], aij[:], tmp[:],
                                    mybir.AluOpType.add)
                        nc.vector.tensor_tensor(
                            wxy[:], WTS[(0, i)][:], WTS[(1, j)][:],
                            mybir.AluOpType.mult)
                        wxyb = wxy[:].unsqueeze(1).broadcast_to(
                            [Y, 3, cw, Z])
                        if first_pair:
                            nc.vector.tensor_tensor(
                                pacc[:], aij[:], wxyb, mybir.AluOpType.mult)
                            first_pair = False
                        else:
                            nc.vector.tensor_tensor(
                                tmp[:], aij[:], wxyb, mybir.AluOpType.mult)
                            nc.vector.tensor_tensor(
                                pacc[:], pacc[:], tmp[:],
                                mybir.AluOpType.add)

                nc.vector.tensor_tensor(
                    pacc[:], pacc[:], T0[:, :, h:h + cw, 2:2 + Z],
                    mybir.AluOpType.add)

                if last:
                    nc.sync.dma_start(out=OUT[:, :, xo:xo + cw, :],
                                      in_=pacc[:])
                else:
                    xw = 2 + xo
                    nc.sync.dma_start(out=W[:, :, xw:xw + cw, 2:2 + Z],
                                      in_=pacc[:])
                    # z wrap halo columns
                    nc.sync.dma_start(out=W[:, :, xw:xw + cw, 0:2],
                                      in_=pacc[:, :, :, Z - 2:Z])
                    nc.sync.dma_start(out=W[:, :, xw:xw + cw, Z + 2:ZP],
                                      in_=pacc[:, :, :, 0:2])

        # steps 0-6 (h=1) share one pool scope (same tags/sizes -> no
        # inter-step pool barriers); step 7 (h=2) gets its own layout.
        with tc.tile_pool(name="main_h1", bufs=1) as pool, \
             tc.tile_pool(name="wpool_h1", bufs=1) as wpool:
            for s in range(STEPS - 1):
                pe = ((lambda ss: lambda: emit_exchange_select(ss))(s - 1)
                      if s > 0 else None)
                emit_step(s, pool, wpool, cxs=cx, tbufs=2, pre_edge=pe)
                emit_exchange_pack(s)
        with tc.tile_pool(name="main_h2", bufs=1) as pool, \
             tc.tile_pool(name="wpool_h2", bufs=1) as wpool:
            emit_step(STEPS - 1, pool, wpool, cxs=8, tbufs=2, wbufs=1,
                      t1bufs=1, kbufs=1,
                      pre_edge=lambda: emit_exchange_select(STEPS - 2))

    nc.finalize()
    _fix_multiwaits(nc)
    return nc


# --------------------------------------------------------------------------
class _Runner:
    def __init__(self, nc, n_cores=8):
        import jax
        from jax.sharding import Mesh, PartitionSpec
        from jax.experimental.shard_map import shard_map
        from concourse import mybir
        from concourse.bass2jax import (_bass_exec_p, install_neuronx_cc_hook,
                                        partition_id_tensor)
        install_neuronx_cc_hook()
        self.jax = jax
        self.n_cores = n_cores
        partition_name = (nc.partition_id_tensor.name
                          if nc.partition_id_tensor else None)
        in_names, out_names, out_avals, zero_outs = [], [], [], []
        for alloc in nc.m.functions[0].allocations:
            if not isinstance(alloc, mybir.MemoryLocationSet):
                continue
            name = alloc.memorylocations[0].name
            if alloc.kind == "ExternalInput":
                if name != partition_name:
                    in_names.append(name)
            elif alloc.kind == "ExternalOutput":
                out_names.append(name)
                shape = tuple(alloc.tensor_shape)
                dtype = mybir.dt.np(alloc.dtype)
                out_avals.append(jax.core.ShapedArray(shape, dtype))
                zero_outs.append(np.zeros(shape, dtype))
        self.in_names, self.out_names = in_names, out_names
        self.out_avals, self.zero_outs = out_avals, zero_outs
        n_params, n_outs = len(in_names), len(out_avals)
        all_in = in_names + out_names + ([partition_name] if partition_name else [])

        def _body(*args):
            operands = list(args)
            if partition_name is not None:
                operands.append(partition_id_tensor())
            outs = _bass_exec_p.bind(
                *operands, out_avals=tuple(out_avals), in_names=tuple(all_in),
                out_names=tuple(out_names), lowering_input_output_aliases=(),
                sim_require_finite=True, sim_require_nnan=True, nc=nc)
            return tuple(outs)

        devices = jax.devices()[:n_cores]
        self.mesh = Mesh(np.asarray(devices), ("core",))
        self.P = PartitionSpec
        in_specs = (PartitionSpec("core"),) * (n_params + n_outs)
        out_specs = (PartitionSpec("core"),) * n_outs
        self.fn = jax.jit(
            shard_map(_body, mesh=self.mesh, in_specs=in_specs,
                      out_specs=out_specs, check_rep=False),
            donate_argnums=tuple(range(n_params, n_params + n_outs)),
            keep_unused=True)
        self.n_params = n_params

    def __call__(self, in_maps):
        from jax.sharding import NamedSharding
        sh = NamedSharding(self.mesh, self.P("core"))
        per_core = [[np.asarray(m[n]) for n in self.in_names] for m in in_maps]
        concat_in = [self.jax.device_put(
            np.concatenate([per_core[c][i] for c in range(self.n_cores)], axis=0),
            sh) for i in range(self.n_params)]
        zeros = [self.jax.device_put(
            np.zeros((self.n_cores * z.shape[0], *z.shape[1:]), z.dtype), sh)
            for z in self.zero_outs]
        out_arrs = self.fn(*concat_in, *zeros)
        self.jax.block_until_ready(out_arrs)
        return [
            {n: np.asarray(out_arrs[i]).reshape(self.n_cores,
                                                *self.out_avals[i].shape)[c]
             for i, n in enumerate(self.out_names)}
            for c in range(self.n_cores)
        ]


def _host_inputs(v):
    maps = []
    vs = (np.asarray(v, dtype=np.float32) * (2.0 ** -STEPS))
    for d in range(8):
        b, q = d // 4, d % 4
        xs = np.arange(32 * q - 2, 32 * q + SLAB + 2) % 128
        sl = vs[b][:, xs, :, :]                      # [3, XW, Y, Z]
        sl = np.transpose(sl, (2, 0, 1, 3))          # [Y, 3, XW, Z]
        sl = np.concatenate([sl[..., Z - 2:Z], sl, sl[..., 0:2]], axis=-1)
        nbr = np.zeros((Y, 2, 4), np.float16)
        nbr[:, 0, (q - 1) % 4] = 1.0
        nbr[:, 1, (q + 1) % 4] = 1.0
        maps.append({"v": np.ascontiguousarray(sl).astype(np.float16),
                     "nbr": nbr})
    return maps


def _get_runner():
    if "r" not in _CACHE:
        _CACHE["r"] = _Runner(_build_kernel())
    return _CACHE["r"]


def kernel(v):
    """v: [2, 3, 128, 128, 128] float32 -> phi: same shape."""
    v = np.asarray(v, dtype=np.float32)
    r = _get_runner()
    res = r(_host_inputs(v))
    out = np.zeros((2, 3, 128, 128, 128), np.float32)
    for d in range(8):
        b, q = d // 4, d % 4
        o = res[d]["out"].astype(np.float32)          # [Y, 3, SLAB, Z]
        out[b][:, 32 * q:32 * q + 32, :, :] = np.transpose(o, (1, 2, 0, 3))
    return out


# revision 18
# speedup vs baseline: 1.0729x; 1.0729x over previous
"""Trainium2 Bass kernel: scaling-and-squaring exponential of a stationary
velocity field (phi <- phi + trilinear_pull(phi, grid + phi), 8 steps, wrap).

Strategy (self-contained; shapes hardcoded for v: [2, 3, 128, 128, 128] f32):
  - 8 NeuronCores = 2 batches x 4 x-slabs (32 planes each). After each step,
    x-halo planes are exchanged with slab neighbors via an AllGather of the
    edge planes over the 4-slab replica group (masks select the two
    neighbors; the mask one-hots are a per-device host input, keeping the
    SPMD program rank-independent). No recompute halo.
  - All device tensors fp16 (DVE tensor_tensor runs 2x for 16-bit dtypes;
    fp16's 11-bit mantissa keeps the 8-step accumulated error ~4x below
    bf16). Host pre-scales v by 2^-STEPS and lays out
    [y=128(part), c=3, x(32+4), z+4(wrap)] fp16; host converts the fp16
    output back to f32.
  - Each step computes the dense masked-tap trilinear form:
      out = sum_{i,j,k} hat(dx-i)*hat(dy-j)*hat(dz-k) * phi[x+i, y+j, z+k]
    with hat(t) = relu(1-|t|) built by ScalarE activation pairs (Abs, Relu
    with affine pre-scale); the z-axis weights are written channel-expanded
    by the Relu (a stride-0 broadcast operand costs +26% on DVE TT). x/z
    taps are free-dim AP offsets; y taps load partition-shifted tiles
    straight from DRAM; odd z offsets read from a z-shifted tile copy
    (ScalarE) so every fp16 TT op stays 4B-aligned (2x mode).
"""
import numpy as np

Y = 128
Z = 128
ZP = Z + 4
STEPS = 8
HS = [1, 1, 1, 1, 1, 1, 1, 2]
SLAB = 32
XW = SLAB + 4          # owned cols at [2, 34); up to 2 halo cols each side
CHUNK_ORDER = [8, 16, 0, 24]   # middle chunks first: they depend only on
                               # owned data, giving the previous step's halo
                               # exchange a full middle-chunk window to land
                               # before the edge chunks consume it

_CACHE = {}


def _fix_multiwaits(nc):
    """This walrus accepts one sync-wait per instruction; split extras onto
    preceding same-engine NoOps."""
    from concourse import mybir
    f = nc.m.functions[0]
    for bb in f.blocks:
        il = bb.instructions
        i = 0
        while i < len(il):
            ins = il[i]
            si = getattr(ins, "sync_info", None)
            if si is None:
                i += 1
                continue
            waits = list(si.on_wait)
            if len(waits) <= 1:
                i += 1
                continue
            for k, w in enumerate(waits[:-1]):
                nop = mybir.InstNoOp(name=f"{ins.name}_w{k}", ins=[], outs=[])
                nop.engine = ins.engine
                nop.sync_info = mybir.SyncInfo(on_wait=[w], on_update=[])
                il.insert(i, nop)
                i += 1
            si.on_wait = [waits[-1]]
            i += 1


def _build_kernel(cx=8):
    from concourse import bacc, mybir, tile
    from contextlib import ExitStack
    F16 = mybir.dt.float16
    ACT = mybir.ActivationFunctionType
    nc = bacc.Bacc("TRN2", target_bir_lowering=False, debug=False, num_devices=8)

    # const APs for activation biases (hat-weight tap offsets)
    F32 = mybir.dt.float32
    for val in (-2.0, -1.0, 2.0):
        t = nc.alloc_sbuf_tensor(f"const-f32-{val}", [128, 1], F32)
        nc.gpsimd.memset(t.ap(), val)
        nc.const_aps.aps[(F32, val)] = t.ap()
    nc.all_engine_barrier()

    # host-prepared: [y, c, x(36), z(wrap-padded)], fp16, scaled 2^-8
    VD = nc.dram_tensor("v", [Y, 3, XW, ZP], F16, kind="ExternalInput")
    # per-device neighbor one-hots: [y, {left,right}, group-rank]
    NBR = nc.dram_tensor("nbr", [Y, 2, 4], F16, kind="ExternalInput")
    OUT = nc.dram_tensor("out", [Y, 3, SLAB, Z], F16, kind="ExternalOutput")

    groups = [[0, 1, 2, 3], [4, 5, 6, 7]]

    with tile.TileContext(nc) as tc, ExitStack() as stack:
        dpool = stack.enter_context(tc.tile_pool(name="dram", bufs=1, space="DRAM"))
        PB = dpool.tile([Y, 3, XW, ZP], F16, tag="pb")
        PC = dpool.tile([Y, 3, XW, ZP], F16, tag="pc")
        npool = stack.enter_context(tc.tile_pool(name="nbrp", bufs=1))
        NBRsb = npool.tile([Y, 2, 4], F16, tag="nbr")
        nc.sync.dma_start(out=NBRsb[:], in_=NBR[:])

        bufs = [None, PB, PC]

        def emit_exchange(s, pool):
            """After step s: swap h'-wide x-edges with slab neighbors."""
            hp = HS[s + 1]
            W = bufs[1 + s % 2]
            ein = dpool.tile([Y, 3, 2 * hp, ZP], F16, tag=f"ein{s}")
            eall = dpool.tile([4 * Y, 3, 2 * hp, ZP], F16, tag=f"eall{s}")
            nc.sync.dma_start(out=ein[:, :, 0:hp], in_=W[:, :, 2:2 + hp])
            nc.sync.dma_start(out=ein[:, :, hp:2 * hp],
                              in_=W[:, :, 2 + SLAB - hp:2 + SLAB])
            nc.gpsimd.collective_compute(
                "AllGather", mybir.AluOpType.bypass, replica_groups=groups,
                ins=[ein[:]], outs=[eall[:]])
            E = []
            for g in range(4):
                e = pool.tile([Y, 3, 2 * hp, ZP], F16, tag=f"ex{g}", bufs=1,
                              name=f"ex{g}")
                nc.sync.dma_start(out=e[:], in_=eall[g * Y:(g + 1) * Y])
                E.append(e)
            HL = pool.tile([Y, 3, hp, ZP], F16, tag="hl", bufs=1, name="hl")
            HR = pool.tile([Y, 3, hp, ZP], F16, tag="hr", bufs=1, name="hr")
            for side, H, zsl in ((0, HL, slice(hp, 2 * hp)),
                                 (1, HR, slice(0, hp))):
                for g in range(4):
                    m = NBRsb[:, side, g:g + 1]
                    if g == 0:
                        nc.vector.scalar_tensor_tensor(
                            H[:], E[g][:, :, zsl], m, E[g][:, :, zsl],
                            op0=mybir.AluOpType.mult, op1=mybir.AluOpType.bypass)
                    else:
                        nc.vector.scalar_tensor_tensor(
                            H[:], E[g][:, :, zsl], m, H[:],
                            op0=mybir.AluOpType.mult, op1=mybir.AluOpType.add)
            nc.sync.dma_start(out=W[:, :, 2 - hp:2], in_=HL[:])
            nc.sync.dma_start(out=W[:, :, 2 + SLAB:2 + SLAB + hp], in_=HR[:])

        def emit_step(s, pool, wpool, cxs, tbufs, wbufs=2, t1bufs=None,
                      kbufs=2):
            R = VD if s == 0 else bufs[1 + (s + 1) % 2]
            W = bufs[1 + s % 2]
            h = HS[s]
            last = (s == STEPS - 1)

            chunks = ([xo for xo in CHUNK_ORDER if xo < SLAB]
                      if cxs == 8 else list(range(0, SLAB, cxs)))
            for xo in chunks:
                cw = min(cxs, SLAB - xo)
                cwi = cw + 2 * h
                xb = 2 + xo - h       # input read base in buffer coords
                # ---- load y-shifted tiles; build z-shifted variants ----
                T = {}
                for j in range(-h, h + 1):
                    t0 = pool.tile([Y, 3, cwi, ZP], F16, tag=f"T{j}_0",
                                   bufs=(tbufs if abs(j) <= 1 else 1),
                                   name=f"t{j}_0")
                    if j == 0:
                        nc.sync.dma_start(out=t0[:],
                                          in_=R[:, :, xb:xb + cwi, :])
                    elif j > 0:
                        nc.sync.dma_start(out=t0[0:Y - j],
                                          in_=R[j:Y, :, xb:xb + cwi, :])
                        nc.sync.dma_start(out=t0[Y - j:Y],
                                          in_=R[0:j, :, xb:xb + cwi, :])
                    else:
                        nc.sync.dma_start(out=t0[-j:Y],
                                          in_=R[0:Y + j, :, xb:xb + cwi, :])
                        nc.sync.dma_start(out=t0[0:-j],
                                          in_=R[Y + j:Y, :, xb:xb + cwi, :])
                    t1 = pool.tile([Y, 3, cwi, ZP], F16, tag=f"T{j}_1",
                                   bufs=(t1bufs or tbufs), name=f"t{j}_1")
                    nc.scalar.copy(t1[:, :, :, 0:ZP - 1], t0[:, :, :, 1:ZP])
                    T[j] = (t0, t1)

                # ---- hat weights on ScalarE: w = relu(1 - |d - i|) ----
                T0 = T[0][0]
                WTS = {}
                for ax, axn in ((0, 'x'), (1, 'y'), (2, 'z')):
                    d = T0[:, ax, h:h + cw, 2:2 + Z]
                    for o in range(-h, h + 1):
                        ab = wpool.tile([Y, cw, Z], F16, bufs=1,
                                        tag=f"ab{axn}", name=f"ab{axn}")
                        nc.scalar.activation(ab[:], d, ACT.Abs,
                                             bias=float(-o), scale=1.0)
                        if ax == 2:
                            # expand across channels at the Relu (ScalarE is
                            # mostly idle; a stride-0 operand costs +26% on
                            # DVE TT, so the 9 consumers want a real tensor)
                            wt = wpool.tile([Y, 3, cw, Z], F16, bufs=wbufs,
                                            tag=f"w{axn}_{o}",
                                            name=f"w{axn}_{o}")
                            abb = ab[:].unsqueeze(1).broadcast_to(
                                [Y, 3, cw, Z])
                            nc.scalar.activation(wt[:], abb, ACT.Relu,
                                                 bias=1.0, scale=-1.0)
                        else:
                            wt = wpool.tile([Y, cw, Z], F16, bufs=1,
                                            tag=f"w{axn}_{o}",
                                            name=f"w{axn}_{o}")
                            nc.scalar.activation(wt[:], ab[:], ACT.Relu,
                                                 bias=1.0, scale=-1.0)
                        WTS[(ax, o)] = wt

                # ---- dense tap accumulation on DVE (all fp16, 2x) ----
                pacc = wpool.tile([Y, 3, cw, Z], F16, tag="pacc",
                                  bufs=kbufs, name="pacc")
                aij = wpool.tile([Y, 3, cw, Z], F16, bufs=kbufs,
                                 tag="aij", name="aij")
                tmp = wpool.tile([Y, 3, cw, Z], F16, bufs=kbufs,
                                 tag="tmp", name="tmp")
                wxy = wpool.tile([Y, cw, Z], F16, bufs=1,
                                 tag="wxy", name="wxy")
                first_pair = True
                for i in range(-h, h + 1):
                    for j in range(-h, h + 1):
                        for ki, k in enumerate(range(-h, h + 1)):
                            zv = (2 + k) % 2  # odd offset -> shifted tile
                            zoff = (2 + k) - zv
                            src = T[j][zv][:, :, h + i:h + i + cw,
                                           zoff:zoff + Z]
                            wzb = WTS[(2, k)][:]
                            if ki == 0:
                                nc.vector.tensor_tensor(
                                    aij[:], src, wzb, mybir.AluOpType.mult)
                            else:
                                nc.vector.tensor_tensor(
                                    tmp[:], src, wzb, mybir.AluOpType.mult)
                                nc.vector.tensor_tensor(
                                    aij[:], aij[:], tmp[:],
                                    mybir.AluOpType.add)
                        nc.vector.tensor_tensor(
                            wxy[:], WTS[(0, i)][:], WTS[(1, j)][:],
                            mybir.AluOpType.mult)
                        wxyb = wxy[:].unsqueeze(1).broadcast_to(
                            [Y, 3, cw, Z])
                        if first_pair:
                            nc.vector.tensor_tensor(
                                pacc[:], aij[:], wxyb, mybir.AluOpType.mult)
                            first_pair = False
                        else:
                            nc.vector.tensor_tensor(
                                tmp[:], aij[:], wxyb, mybir.AluOpType.mult)
                            nc.vector.tensor_tensor(
                                pacc[:], pacc[:], tmp[:],
                                mybir.AluOpType.add)

                nc.vector.tensor_tensor(
                    pacc[:], pacc[:], T0[:, :, h:h + cw, 2:2 + Z],
                    mybir.AluOpType.add)

                if last:
                    nc.sync.dma_start(out=OUT[:, :, xo:xo + cw, :],
                                      in_=pacc[:])
                else:
                    xw = 2 + xo
                    nc.sync.dma_start(out=W[:, :, xw:xw + cw, 2:2 + Z],
                                      in_=pacc[:])
                    # z wrap halo columns
                    nc.sync.dma_start(out=W[:, :, xw:xw + cw, 0:2],
                                      in_=pacc[:, :, :, Z - 2:Z])
                    nc.sync.dma_start(out=W[:, :, xw:xw + cw, Z + 2:ZP],
                                      in_=pacc[:, :, :, 0:2])

        # steps 0-6 (h=1) share one pool scope (same tags/sizes -> no
        # inter-step pool barriers); step 7 (h=2) gets its own layout.
        with tc.tile_pool(name="main_h1", bufs=1) as pool, \
             tc.tile_pool(name="wpool_h1", bufs=1) as wpool:
            for s in range(STEPS - 1):
                emit_step(s, pool, wpool, cxs=cx, tbufs=2)
                emit_exchange(s, wpool)
        with tc.tile_pool(name="main_h2", bufs=1) as pool, \
             tc.tile_pool(name="wpool_h2", bufs=1) as wpool:
            emit_step(STEPS - 1, pool, wpool, cxs=8, tbufs=2, wbufs=1,
                      t1bufs=1, kbufs=1)

    nc.finalize()
    _fix_multiwaits(nc)
    return nc


# --------------------------------------------------------------------------
class _Runner:
    def __init__(self, nc, n_cores=8):
        import jax
        from jax.sharding import Mesh, PartitionSpec
        from jax.experimental.shard_map import shard_map
        from concourse import mybir
        from concourse.bass2jax import (_bass_exec_p, install_neuronx_cc_hook,
                                        partition_id_tensor)
        install_neuronx_cc_hook()
        self.jax = jax
        self.n_cores = n_cores
        partition_name = (nc.partition_id_tensor.name
                          if nc.partition_id_tensor else None)
        in_names, out_names, out_avals, zero_outs = [], [], [], []
        for alloc in nc.m.functions[0].allocations:
            if not isinstance(alloc, mybir.MemoryLocationSet):
                continue
            name = alloc.memorylocations[0].name
            if alloc.kind == "ExternalInput":
                if name != partition_name:
                    in_names.append(name)
            elif alloc.kind == "ExternalOutput":
                out_names.append(name)
                shape = tuple(alloc.tensor_shape)
                dtype = mybir.dt.np(alloc.dtype)
                out_avals.append(jax.core.ShapedArray(shape, dtype))
                zero_outs.append(np.zeros(shape, dtype))
        self.in_names, self.out_names = in_names, out_names
        self.out_avals, self.zero_outs = out_avals, zero_outs
        n_params, n_outs = len(in_names), len(out_avals)
        all_in = in_names + out_names + ([partition_name] if partition_name else [])

        def _body(*args):
            operands = list(args)
            if partition_name is not None:
                operands.append(partition_id_tensor())
            outs = _bass_exec_p.bind(
                *operands, out_avals=tuple(out_avals), in_names=tuple(all_in),
                out_names=tuple(out_names), lowering_input_output_aliases=(),
                sim_require_finite=True, sim_require_nnan=True, nc=nc)
            return tuple(outs)

        devices = jax.devices()[:n_cores]
        self.mesh = Mesh(np.asarray(devices), ("core",))
        self.P = PartitionSpec
        in_specs = (PartitionSpec("core"),) * (n_params + n_outs)
        out_specs = (PartitionSpec("core"),) * n_outs
        self.fn = jax.jit(
            shard_map(_body, mesh=self.mesh, in_specs=in_specs,
                      out_specs=out_specs, check_rep=False),
            donate_argnums=tuple(range(n_params, n_params + n_outs)),
            keep_unused=True)
        self.n_params = n_params

    def __call__(self, in_maps):
        from jax.sharding import NamedSharding
        sh = NamedSharding(self.mesh, self.P("core"))
        per_core = [[np.asarray(m[n]) for n in self.in_names] for m in in_maps]
        concat_in = [self.jax.device_put(
            np.concatenate([per_core[c][i] for c in range(self.n_cores)], axis=0),
            sh) for i in range(self.n_params)]
        zeros = [self.jax.device_put(
            np.zeros((self.n_cores * z.shape[0], *z.shape[1:]), z.dtype), sh)
            for z in self.zero_outs]
        out_arrs = self.fn(*concat_in, *zeros)
        self.jax.block_until_ready(out_arrs)
        return [
            {n: np.asarray(out_arrs[i]).reshape(self.n_cores,
                                                *self.out_avals[i].shape)[c]
             for i, n in enumerate(self.out_names)}
            for c in range(self.n_cores)
        ]


def _host_inputs(v):
    maps = []
    vs = (np.asarray(v, dtype=np.float32) * (2.0 ** -STEPS))
    for d in range(8):
        b, q = d // 4, d % 4
        xs = np.arange(32 * q - 2, 32 * q + SLAB + 2) % 128
        sl = vs[b][:, xs, :, :]                      # [3, XW, Y, Z]
        sl = np.transpose(sl, (2, 0, 1, 3))          # [Y, 3, XW, Z]
        sl = np.concatenate([sl[..., Z - 2:Z], sl, sl[..., 0:2]], axis=-1)
        nbr = np.zeros((Y, 2, 4), np.float16)
        nbr[:, 0, (q - 1) % 4] = 1.0
        nbr[:, 1, (q + 1) % 4] = 1.0
        maps.append({"v": np.ascontiguousarray(sl).astype(np.float16),
                     "nbr": nbr})
    return maps


def _get_runner():
    if "r" not in _CACHE:
        _CACHE["r"] = _Runner(_build_kernel())
    return _CACHE["r"]


def kernel(v):
    """v: [2, 3, 128, 128, 128] float32 -> phi: same shape."""
    v = np.asarray(v, dtype=np.float32)
    r = _get_runner()
    res = r(_host_inputs(v))
    out = np.zeros((2, 3, 128, 128, 128), np.float32)
    for d in range(8):
        b, q = d // 4, d % 4
        o = res[d]["out"].astype(np.float32)          # [Y, 3, SLAB, Z]
        out[b][:, 32 * q:32 * q + 32, :, :] = np.transpose(o, (1, 2, 0, 3))
    return out
